# revision 1
# baseline (speedup 1.0000x reference)
"""GPS layer (GCN + dense Performer attention + FFN) on 8 Trainium2 cores.

Strategy (per core, rows R=1024 of N=8192 nodes):
  - GCN segment-sum as a dense matmul with the normalized adjacency
    A = D^-1/2 (Adj + I) D^-1/2, built host-side, shipped fp8-e4m3 in an
    lhsT-friendly layout, and computed as (A @ h) @ W_gcn with h also
    fp8-e4m3 so the big [1024,8192]x[8192,256] matmul runs in DoubleRow
    mode.  A-tiles are prefetched one row-block ahead, alternating between
    the gpsimd and scalar DMA queues, so the PE never starves (and never
    drops to the cold 1.2 GHz HAM clock) during the GCN.
  - Attention in transposed score layout ST[c, r] = kf@qf^T: softmax needs
    no max subtraction (scores bounded), the denominator comes from an
    appended ones-column of V, and exp(ST) tiles are directly the lhsT of
    the P@V matmul.  kf/qf/v are fp8-e3m4 (4 mantissa bits, range +-15.5).
  - k-features and V are all-gathered in ONE packed byte-buffer per half
    (258 KB/rank, 2 collectives total — more, smaller collectives lose to
    the serial CC-stream overhead).  All pack writes are ordered before
    all unpack loads on the sync ring so a waiting load can never dam up a
    later collective's input writes.
  - Phase 2 is issued in arrival order to ride out cross-core launch skew:
    all half-A score chunks for both row slabs, then the half-A partial of
    every P@V accumulation (saved to SBUF), and only then the half-B
    scores + P@V completion + FFN per slab.  ~80us of half-A-only PE work
    covers the skew + collective latency of half B.
  - exp() is batched 2 score-chunks at a time ([128,1024] ACTIVATE);
    free-dim bias adds run on DVE broadcast tiles, not rank-1 PE matmuls.
"""

import os
import sys

sys.path.insert(0, "/opt/trn_rl_repo")
os.environ.setdefault("MYCRO_LOCAL_CACHE", "1")

import numpy as np
import ml_dtypes

import concourse.bass as bass
import concourse.tile as tile
from concourse import bacc, mybir
from concourse.bass_utils import run_bass_kernel_spmd
from concourse.masks import make_identity

f32 = mybir.dt.float32
bf16 = mybir.dt.bfloat16
f8e3 = mybir.dt.float8e3
f8e4 = mybir.dt.float8e4
DR = mybir.MatmulPerfMode.DoubleRow
BF = ml_dtypes.bfloat16
E3 = ml_dtypes.float8_e3m4
E4 = ml_dtypes.float8_e4m3

N, D, F, M = 8192, 256, 512, 256
NCORES = 8
R = N // NCORES          # rows per core (1024)
RB = R // 128            # row blocks per core (8)
KC = D // 128            # feature chunks (2)
NCH = N // 128           # node chunks (64)
FC = F // 128            # ffn chunks (4)
VA = 260                 # v free dim: 256 features + ones col + pad
EPS = 1e-5
RH = R // 2              # rows per collective half (512)
KFT_B = M * RH           # kft bytes per half (e3m4): 131072
VAUG_B = RH * VA         # vaug bytes per half (e3m4): 133120
PACK = KFT_B + VAUG_B    # 264192 bytes per rank per half


def _ln_block(nc, pool, x_sb, out_sb, g_bc=None, be_bc=None, eps_t=None,
              tail=None):
    """LayerNorm over free dim (256) of x_sb [128, 256] f32 -> out_sb.

    With g_bc=None only the normalization is applied (caller handles the
    affine, e.g. off the critical chain on gpsimd).  `tail` selects the
    engine for the gamma/beta ops.
    """
    tail = tail or nc.vector
    stats = pool.tile([128, 6], f32, tag="ln_stats")
    nc.vector.bn_stats(stats[:], x_sb)
    mv = pool.tile([128, 2], f32, tag="ln_mv")
    nc.vector.bn_aggr(mv[:], stats[:])
    # rstd = 1/sqrt(var + eps)
    nc.scalar.activation(mv[:, 1:2], mv[:, 1:2],
                         mybir.ActivationFunctionType.Sqrt, bias=eps_t)
    nc.vector.reciprocal(mv[:, 1:2], mv[:, 1:2])
    nc.vector.tensor_scalar(out=out_sb, in0=x_sb,
                            scalar1=mv[:, 0:1], scalar2=mv[:, 1:2],
                            op0=mybir.AluOpType.subtract,
                            op1=mybir.AluOpType.mult)
    if g_bc is not None:
        tail.tensor_mul(out_sb, out_sb, g_bc)
        tail.tensor_add(out_sb, out_sb, be_bc)


def _build():
    nc = bacc.Bacc("TRN2", target_bir_lowering=False, debug=False,
                   num_devices=NCORES)

    def inp(name, shape, dt):
        return nc.dram_tensor(name, shape, dt, kind="ExternalInput")

    # at[rb,hf,p,k,f] = A[R0+rb*128+f, (hf*32+k)*128+p] — one fully
    # contiguous 512KB region per (rb, half) for full-rate DMA with only
    # two trigger instructions per row block
    at_h = inp("at", [RB, 2, 128, NCH // 2, 128], f8e4)
    hn_h = inp("hn", [128, NCH * D], f8e4)        # h node-major, pre-arranged [p, c*d]
    hres_h = inp("hres", [128, RB * D], f32)      # h rows + b_gcn, pre-arranged
    wgcn_h = inp("wgcn", [D, D], bf16)
    wq_h = inp("wq", [D, D], bf16)
    wk_h = inp("wk", [D, D], bf16)
    wv_h = inp("wv", [D, D], bf16)
    wo_h = inp("wo", [D, D], bf16)
    rft_h = inp("rft", [D, M], bf16)              # RF^T
    w1_h = inp("w1", [D, F], bf16)
    w2_h = inp("w2", [F, D], bf16)
    bq_h = inp("bqc", [D, 1], f32)
    bk_h = inp("bkc", [D, 1], f32)
    b1c_h = inp("b1c", [F, 1], f32)
    bvr_h = inp("bvr", [1, D], bf16)
    bor_h = inp("bor", [1, D], bf16)
    b2r_h = inp("b2r", [1, D], bf16)
    gb_h = {}
    for nm in ("g1", "be1", "g2", "be2", "g3", "be3"):
        gb_h[nm] = inp(nm, [1, D], bf16)

    out_h = nc.dram_tensor("out", [R, D], f32, kind="ExternalOutput")

    with tile.TileContext(nc) as tc:
        _body(tc, at_h, hn_h, hres_h, wgcn_h, wq_h, wk_h, wv_h, wo_h, rft_h,
              w1_h, w2_h, bq_h, bk_h, b1c_h, bvr_h, bor_h, b2r_h, gb_h, out_h)
    nc.compile()
    return nc


def _body(tc, at_h, hn_h, hres_h, wgcn_h, wq_h, wk_h, wv_h, wo_h, rft_h,
          w1_h, w2_h, bq_h, bk_h, b1c_h, bvr_h, bor_h, b2r_h, gb_h, out_h):
    from contextlib import ExitStack
    nc = tc.nc
    Exp = mybir.ActivationFunctionType.Exp
    Copy = mybir.ActivationFunctionType.Copy
    Relu = mybir.ActivationFunctionType.Relu

    with ExitStack() as octx:
        const = octx.enter_context(tc.tile_pool(name="const", bufs=1))
        persist = octx.enter_context(tc.tile_pool(name="persist", bufs=1))
        dram = octx.enter_context(tc.tile_pool(name="dram", bufs=1, space="DRAM"))

        # ---- persistent activations ----
        kfa_sb = persist.tile([128, KC, N], f8e3, tag="kfa")
        vaug_sb = persist.tile([128, NCH, VA], f8e3, tag="vaug")
        h1_sb = persist.tile([128, RB, D], f32, tag="h1")
        qft_sb = persist.tile([128, KC, R], f8e3, tag="qft")

        # ---- packed collective DRAM buffers (two halves) ----
        pack_loc = [dram.tile([PACK], f8e3, tag=f"pk_loc{h}", name=f"pk_loc{h}")
                    for h in range(2)]
        pack_all = [dram.tile([NCORES * PACK], f8e3, tag=f"pk_all{h}",
                              name=f"pk_all{h}", addr_space="Shared")
                    for h in range(2)]

        # ---- phase-1 input loads: hn/hres first (gate the first matmul),
        #      then weights in first-use order, all on the sync ring ----
        def wtile(h, chunks, width, name):
            t = const.tile([128, chunks, width], bf16, tag=name)
            nc.sync.dma_start(
                t[:], h[:, :].rearrange("(c p) w -> p c w", p=128))
            return t

        def bcast_load(h, width, dt, name):
            t = const.tile([128, width], dt, tag=name)
            bc = bass.AP(tensor=h.ap().tensor, offset=h.ap().offset,
                         ap=[[0, 128]] + list(h.ap().ap[1:]))
            nc.sync.dma_start(t[:], bc)
            return t

        gcn_p = tc.tile_pool(name="gcn", bufs=1)
        gcn = gcn_p.__enter__()
        hn_sb = gcn.tile([128, NCH, D], f8e4, tag="hn")
        hres_sb = gcn.tile([128, RB, D], f32, tag="hres")

        def hn_load(sl, eng):
            w = NCH * D // 4
            eng.dma_start(
                hn_sb[:].rearrange("p c d -> p (c d)")[:, sl * w:(sl + 1) * w],
                hn_h[:, sl * w:(sl + 1) * w])

        hn_load(0, nc.sync)
        nc.sync.dma_start(hres_sb[:, 0, :], hres_h[:, 0:D])
        wgcn_sb = wtile(wgcn_h, KC, D, "wgcn")
        gb_sb = {}
        for nm in ("g1", "be1"):
            gb_sb[nm] = bcast_load(gb_h[nm], D, bf16, nm)
        hn_load(3, nc.sync)
        for rb in range(1, RB):
            nc.sync.dma_start(hres_sb[:, rb, :], hres_h[:, rb * D:(rb + 1) * D])
        wk_sb = wtile(wk_h, KC, D, "wk")
        wq_sb = wtile(wq_h, KC, D, "wq")
        wv_sb = wtile(wv_h, KC, D, "wv")
        rft_sb = wtile(rft_h, KC, M, "rft")
        bq_sb = const.tile([128, KC], f32, tag="bq")
        bk_sb = const.tile([128, KC], f32, tag="bk")
        for j in range(KC):
            nc.sync.dma_start(bq_sb[:, j:j + 1], bq_h[j * 128:(j + 1) * 128, :])
            nc.sync.dma_start(bk_sb[:, j:j + 1], bk_h[j * 128:(j + 1) * 128, :])
        bvr_bc = bcast_load(bvr_h, D, bf16, "bvr")
        ones_k1 = const.tile([1, 128], bf16, tag="ones")
        nc.vector.memset(ones_k1[:], 1.0)
        ident_bf = const.tile([128, 128], bf16, tag="ident")
        make_identity(nc, ident_bf[:])
        eps_t = const.tile([128, 1], f32, tag="eps")
        nc.vector.memset(eps_t[:], EPS)

        def load_p2_weights():
            # phase-2-only weights: loaded at the tail of phase 1 so they
            # never sit ahead of at-tile or pack traffic on the sync ring
            w = {}
            w["wo"] = wtile(wo_h, KC, D, "wo")
            w["w1"] = wtile(w1_h, KC, F, "w1")
            w["w2"] = wtile(w2_h, FC, D, "w2")
            bor_r = const.tile([1, D], bf16, tag="bor")
            nc.sync.dma_start(bor_r[:], bor_h[:, :])
            w["bor"] = bor_r
            b1c_sb = const.tile([128, FC], f32, tag="b1c")
            for jf in range(FC):
                nc.sync.dma_start(b1c_sb[:, jf:jf + 1],
                                  b1c_h[jf * 128:(jf + 1) * 128, :])
            w["b1c"] = b1c_sb
            b2r_r = const.tile([1, D], bf16, tag="b2r")
            nc.sync.dma_start(b2r_r[:], b2r_h[:, :])
            w["b2r"] = b2r_r
            for nm in ("g2", "be2", "g3", "be3"):
                gb_sb[nm] = bcast_load(gb_h[nm], D, bf16, nm)
            return w

        def fire(half):
            nc.gpsimd.collective_compute(
                "AllGather", mybir.AluOpType.bypass,
                replica_groups=[list(range(NCORES))],
                ins=[pack_loc[half][:].opt()], outs=[pack_all[half][:].opt()])

        def load_kfa(half):
            # per-rank DMAs so the first score chunks can start before the
            # whole unpack lands
            r0 = half * RH
            for c in range(NCORES):
                for mc in range(KC):
                    off = c * PACK + mc * 128 * RH
                    src = pack_all[half][off:off + 128 * RH].rearrange(
                        "(p r) -> p r", p=128)
                    nc.sync.dma_start(
                        kfa_sb[:, mc, c * R + r0:c * R + r0 + RH], src)

        def load_vaug(half):
            for c in range(NCORES):
                cg0 = c * 8 + half * 4
                off = c * PACK + KFT_B
                src = pack_all[half][off:off + VAUG_B].rearrange(
                    "(l p v) -> p l v", p=128, v=VA)
                nc.sync.dma_start(vaug_sb[:, cg0:cg0 + 4, :], src)

        # ============ Phase 1: GCN + qkv, packed collective per half ========
        kt_sb = gcn.tile([128, KC, R], bf16, tag="kt")
        qt_sb = gcn.tile([128, KC, R], bf16, tag="qt")
        kft_sb = gcn.tile([128, KC, R], f8e3, tag="kft")
        h1t_sb = gcn.tile([128, KC, R], bf16, tag="h1t")
        with ExitStack() as p1:
            atp = p1.enter_context(tc.tile_pool(name="atp", bufs=6))
            sc1 = p1.enter_context(tc.tile_pool(name="sc1", bufs=4))
            mm_ps = p1.enter_context(tc.tile_pool(name="mm_ps", bufs=2, space="PSUM"))
            ah_ps = p1.enter_context(tc.tile_pool(name="ah_ps", bufs=3, space="PSUM"))
            tp_ps = p1.enter_context(tc.tile_pool(name="tp_ps", bufs=2, space="PSUM"))

            AT_Q = (nc.gpsimd, nc.scalar, nc.sync)

            def at_load(rb):
                """Prefetch both A-halves of row block rb, one 512KB DMA per
                queue.  atp bufs >= 2 row blocks, so these triggers never
                wait for pool space and cannot dam a ring.  Middle blocks
                also use the sync queue (idle between the input loads and
                the pack writes) for a 3-queue stream."""
                tiles = []
                for hf in range(2):
                    at_t = atp.tile([128, NCH // 2, 128], f8e4, tag="at",
                                    name=f"at{rb}_{hf}")
                    if 2 <= rb <= 4:
                        q = AT_Q[(2 * rb + hf) % 3]
                    else:
                        q = AT_Q[hf]
                    q.dma_start(at_t[:], at_h[rb, hf])
                    tiles.append(at_t)
                if rb == 0:
                    # the hn mid-slabs ride behind the first at transfers
                    hn_load(1, nc.gpsimd)
                    hn_load(2, nc.scalar)
                return tiles

            def a_h(rb, tiles):
                """A @ h for row block rb: fp8 DoubleRow accumulation."""
                ps = ah_ps.tile([128, D], f32, tag="ah")
                for hf in range(2):
                    at_t = tiles[hf]
                    for k in range(NCH // 4):
                        pr = hf * (NCH // 4) + k
                        c0 = hf * (NCH // 2) + 2 * k
                        nc.tensor.matmul(
                            ps[:], at_t[:, 2 * k:2 * k + 2, :],
                            hn_sb[:, c0:c0 + 2, :],
                            start=(pr == 0), stop=(pr == NCH // 2 - 1),
                            perf_mode=DR)
                return ps

            # post-GCN work for row block rb, split into three pipeline
            # stages so no cross-engine chain ever blocks the PE queue:
            #  S0(rb):   ahb PSUM->SBUF copy (ACT), hoisted right after
            #            a_h(rb) so the later PE transposes never wait
            #  S1(rb-2): ah transposes + W_gcn matmul + residual + LN
            #  S2(rb-3): h1 transposes into h1t (their DVE inputs settled
            #            a full iteration ago)
            ahb_t = {}
            h1bf_t = {}

            def post_s0(rb, ps):
                ahb = sc1.tile([128, D], bf16, tag="ahb")
                nc.scalar.activation(ahb[:], ps[:], Copy)
                ahb_t[rb] = ahb

            def post_s1(rb):
                ahb = ahb_t.pop(rb)
                ahT = sc1.tile([128, KC, 128], bf16, tag="ahT")
                for j in range(KC):
                    tp = tp_ps.tile([128, 128], bf16, tag="tp1")
                    nc.tensor.transpose(tp[:], ahb[:, j * 128:(j + 1) * 128],
                                        ident_bf[:])
                    nc.scalar.activation(ahT[:, j, :], tp[:], Copy)
                hl = mm_ps.tile([128, 512], f32, tag="mm")
                for j in range(KC):
                    nc.tensor.matmul(hl[:, 0:D], ahT[:, j, :], wgcn_sb[:, j, :],
                                     start=(j == 0), stop=(j == KC - 1))
                x1 = sc1.tile([128, D], f32, tag="x1")
                nc.vector.tensor_add(x1[:], hl[:, 0:D], hres_sb[:, rb, :])
                # normalized h1 feeds q/k/v (g1/be1 folded into Wq/Wk/Wv
                # host-side); affine h1 for the phase-2 residual off-chain
                h1n = sc1.tile([128, D], f32, tag="h1n")
                _ln_block(nc, sc1, x1[:], h1n[:], eps_t=eps_t[:])
                nc.gpsimd.tensor_mul(h1_sb[:, rb, :], h1n[:], gb_sb["g1"][:])
                nc.gpsimd.tensor_add(h1_sb[:, rb, :], h1_sb[:, rb, :],
                                     gb_sb["be1"][:])
                h1bf = sc1.tile([128, D], bf16, tag="h1bf")
                nc.vector.tensor_copy(h1bf[:], h1n[:])
                h1bf_t[rb] = h1bf

            def post_s2(rb):
                h1bf = h1bf_t.pop(rb)
                for j in range(KC):
                    tp = tp_ps.tile([128, 128], bf16, tag="tp1")
                    nc.tensor.transpose(tp[:], h1bf[:, j * 128:(j + 1) * 128],
                                        ident_bf[:])
                    nc.vector.tensor_copy(
                        h1t_sb[:, j, rb * 128:(rb + 1) * 128], tp[:])

            def qkv_half(half):
                r0 = half * RH
                # kT (feature-major) for rows [r0, r0+RH)
                for jj in range(KC):
                    ps = mm_ps.tile([128, 512], f32, tag="mm")
                    for j in range(KC):
                        nc.tensor.matmul(
                            ps[:],
                            wk_sb[:, j, jj * 128:(jj + 1) * 128],
                            h1t_sb[:, j, r0:r0 + RH],
                            start=(j == 0), stop=(j == KC - 1))
                    nc.vector.tensor_scalar(
                        out=kt_sb[:, jj, r0:r0 + RH], in0=ps[:],
                        scalar1=bk_sb[:, jj:jj + 1], scalar2=None,
                        op0=mybir.AluOpType.add)

                def rf_proj(src, dst, store_kft):
                    for mc in range(KC):
                        ps = mm_ps.tile([128, 512], f32, tag="mm")
                        for j in range(KC):
                            nc.tensor.matmul(
                                ps[:],
                                rft_sb[:, j, mc * 128:(mc + 1) * 128],
                                src[:, j, r0:r0 + RH],
                                start=(j == 0), stop=(j == KC - 1))
                        nc.vector.tensor_copy(dst[:, mc, r0:r0 + RH], ps[:])
                        if store_kft:
                            nc.sync.dma_start(
                                pack_loc[half][mc * 128 * RH:
                                               (mc + 1) * 128 * RH].rearrange(
                                    "(p r) -> p r", p=128),
                                dst[:, mc, r0:r0 + RH])

                rf_proj(kt_sb, kft_sb, True)
                # v rows (node-major) + ones column, e3m4 into the pack buffer
                for blk in range(RB // 2):
                    rb = half * (RB // 2) + blk
                    ps = mm_ps.tile([128, 512], f32, tag="mm")
                    for j in range(KC):
                        nc.tensor.matmul(ps[:, 0:D],
                                         h1t_sb[:, j, rb * 128:(rb + 1) * 128],
                                         wv_sb[:, j, :],
                                         start=(j == 0), stop=(j == KC - 1))
                    vt = sc1.tile([128, VA], f8e3, tag="vaug")
                    nc.vector.tensor_add(vt[:, 0:D], ps[:, 0:D], bvr_bc[:])
                    nc.vector.memset(vt[:, D:D + 1], 1.0)
                    nc.vector.memset(vt[:, D + 1:VA], 0.0)
                    off = KFT_B + blk * 128 * VA
                    nc.sync.dma_start(
                        pack_loc[half][off:off + 128 * VA].rearrange(
                            "(p v) -> p v", p=128),
                        vt[:])
                fire(half)
                # qT + random-feature projection of q: local-only, so it
                # runs after the collective trigger
                for jj in range(KC):
                    ps = mm_ps.tile([128, 512], f32, tag="mm")
                    for j in range(KC):
                        nc.tensor.matmul(
                            ps[:],
                            wq_sb[:, j, jj * 128:(jj + 1) * 128],
                            h1t_sb[:, j, r0:r0 + RH],
                            start=(j == 0), stop=(j == KC - 1))
                    nc.vector.tensor_scalar(
                        out=qt_sb[:, jj, r0:r0 + RH], in0=ps[:],
                        scalar1=bq_sb[:, jj:jj + 1], scalar2=None,
                        op0=mybir.AluOpType.add)
                rf_proj(qt_sb, qft_sb, False)

            # GCN row blocks with the 3-stage post pipeline; at-tiles
            # prefetched one block ahead; qkv + collective per half
            tiles_cur = at_load(0)
            s2_done = set()

            def s2(rb):
                if 0 <= rb < RB and rb not in s2_done:
                    s2_done.add(rb)
                    post_s2(rb)

            for rb in range(RB):
                ps = a_h(rb, tiles_cur)
                post_s0(rb, ps)
                if rb >= 2:
                    post_s1(rb - 2)
                s2(rb - 3)
                if rb + 1 < RB:
                    tiles_cur = at_load(rb + 1)
                if rb == 5:
                    # hoist S2(3) (one short PE wait) to fire half A early
                    s2(3)
                    qkv_half(0)
            post_s1(RB - 2)
            s2(RB - 3)
            post_s1(RB - 1)
            s2(RB - 2)
            s2(RB - 1)
            qkv_half(1)
            p2w = load_p2_weights()
            wo_sb, w1_sb, w2_sb = p2w["wo"], p2w["w1"], p2w["w2"]
            bor_r, b1c_sb, b2r_r = p2w["bor"], p2w["b1c"], p2w["b2r"]

            # unpack loads: strictly after every pack write on the sync ring
            for half in range(2):
                load_kfa(half)
                load_vaug(half)

        gcn_p.__exit__(None, None, None)

        # ============ Phase 2: attention + FFN ============
        with ExitStack() as p3:
            slabs = p3.enter_context(tc.tile_pool(name="slabs", bufs=2))
            pva_p = p3.enter_context(tc.tile_pool(name="pva", bufs=1))
            sc3 = p3.enter_context(tc.tile_pool(name="sc3", bufs=2))
            st_ps = p3.enter_context(tc.tile_pool(name="st_ps", bufs=2, space="PSUM"))
            num_ps = p3.enter_context(tc.tile_pool(name="num_ps", bufs=1, space="PSUM"))
            tp2_ps = p3.enter_context(tc.tile_pool(name="tp2_ps", bufs=1, space="PSUM"))
            sm_ps = p3.enter_context(tc.tile_pool(name="sm_ps", bufs=1, space="PSUM"))

            RC = 512  # rows per score slab (2 slabs cover R=1024)
            NSL = R // RC
            half_cgs = [[c * 8 + h * 4 + l for c in range(NCORES)
                         for l in range(4)] for h in range(2)]

            slab = [slabs.tile([128, NCH, RC], bf16, tag="slab",
                               name=f"slab{i}")
                    for i in range(NSL)]
            pva_sb = pva_p.tile([128, RB, VA], bf16, tag="pva")

            def scores_h(rc, half):
                """Score+exp for slab rc, chunks of `half` (32 chunks)."""
                chunks = half_cgs[half]
                for ci in range(0, 32, 2):
                    ps = st_ps.tile([128, 2, RC], f32, tag="st")
                    for t in range(2):
                        cg = chunks[ci + t]
                        for j in range(KC):
                            nc.tensor.matmul(
                                ps[:, t, :],
                                kfa_sb[:, j, cg * 128:(cg + 1) * 128],
                                qft_sb[:, j, rc * RC:(rc + 1) * RC],
                                start=(j == 0), stop=(j == KC - 1))
                    # batched exp over 2 chunks: [128, 1024] ACTIVATE;
                    # raw/16 is the 1/sqrt(D) score scale (max ~15.1, safe
                    # in bf16 without max subtraction)
                    cg0 = chunks[ci]
                    nc.scalar.activation(slab[rc][:, cg0:cg0 + 2, :], ps[:],
                                         Exp, scale=1.0 / 16.0)

            def pv_a(rc):
                """Half-A partial of P@V for the 4 row blocks of slab rc."""
                for hb in range(RC // 128):
                    rb = rc * (RC // 128) + hb
                    nps = num_ps.tile([128, VA], f32, tag="num")
                    for i, cg in enumerate(half_cgs[0]):
                        nc.tensor.matmul(
                            nps[:], slab[rc][:, cg, hb * 128:(hb + 1) * 128],
                            vaug_sb[:, cg, :],
                            start=(i == 0), stop=(i == NCH // 2 - 1))
                    nc.vector.tensor_copy(pva_sb[:, rb, :], nps[:])

            def pv_b_ffn(rc):
                """Half-B P@V + output projection + FFN for slab rc."""
                for hb in range(RC // 128):
                    rb = rc * (RC // 128) + hb
                    nps = num_ps.tile([128, VA], f32, tag="num")
                    for i, cg in enumerate(half_cgs[1]):
                        nc.tensor.matmul(
                            nps[:], slab[rc][:, cg, hb * 128:(hb + 1) * 128],
                            vaug_sb[:, cg, :],
                            start=(i == 0), stop=(i == NCH // 2 - 1))
                    nc.vector.tensor_add(nps[:], nps[:], pva_sb[:, rb, :])
                    rec = sc3.tile([128, 1], f32, tag="rec")
                    nc.vector.reciprocal(rec[:], nps[:, D:D + 1])
                    attn_bf = sc3.tile([128, D], bf16, tag="attn")
                    nc.scalar.activation(attn_bf[:], nps[:, 0:D], Copy,
                                         scale=rec[:, 0:1])
                    attnT = sc3.tile([128, KC, 128], bf16, tag="attnT")
                    for j in range(KC):
                        tp = tp2_ps.tile([128, 128], bf16, tag="tp2")
                        nc.tensor.transpose(
                            tp[:], attn_bf[:, j * 128:(j + 1) * 128], ident_bf[:])
                        nc.vector.tensor_copy(attnT[:, j, :], tp[:])

                    hg = sm_ps.tile([128, D], f32, tag="smz")
                    for j in range(KC):
                        nc.tensor.matmul(hg[:], attnT[:, j, :],
                                         wo_sb[:, j, :],
                                         start=(j == 0), stop=False)
                    nc.tensor.matmul(hg[:], ones_k1[:], bor_r[:],
                                     start=False, stop=True)
                    x2 = sc3.tile([128, D], f32, tag="x2")
                    nc.vector.tensor_add(x2[:], hg[:], h1_sb[:, rb, :])
                    # normalized h2 feeds the FFN (g2/be2 folded into W1/b1
                    # host-side); the affine h2 for the residual is computed
                    # off-chain on gpsimd
                    h2n = sc3.tile([128, D], f32, tag="h2n")
                    _ln_block(nc, sc3, x2[:], h2n[:], eps_t=eps_t[:])
                    h2 = sc3.tile([128, D], f32, tag="h2")
                    nc.gpsimd.tensor_mul(h2[:], h2n[:], gb_sb["g2"][:])
                    nc.gpsimd.tensor_add(h2[:], h2[:], gb_sb["be2"][:])
                    h2bf = sc3.tile([128, D], bf16, tag="h2bf")
                    nc.vector.tensor_copy(h2bf[:], h2n[:])
                    h2T = sc3.tile([128, KC, 128], bf16, tag="h2T")
                    for j in range(KC):
                        tp = tp2_ps.tile([128, 128], bf16, tag="tp2")
                        nc.tensor.transpose(
                            tp[:], h2bf[:, j * 128:(j + 1) * 128], ident_bf[:])
                        nc.vector.tensor_copy(h2T[:, j, :], tp[:])

                    # FFN1 in transposed layout: uT[f, node] directly from
                    # W1-chunk lhsT x h2T, bias+ReLU fused into one ACT op
                    # per chunk (b1 is per-partition here)
                    uT = sc3.tile([128, FC, 128], bf16, tag="uT")
                    for jf in range(FC):
                        up = tp2_ps.tile([128, 128], f32, tag="ups")
                        for j in range(KC):
                            nc.tensor.matmul(
                                up[:], w1_sb[:, j, jf * 128:(jf + 1) * 128],
                                h2T[:, j, :],
                                start=(j == 0), stop=(j == KC - 1))
                        nc.scalar.activation(uT[:, jf, :], up[:], Relu,
                                             bias=b1c_sb[:, jf:jf + 1])

                    o2 = sm_ps.tile([128, D], f32, tag="smz")
                    for jf in range(FC):
                        nc.tensor.matmul(o2[:], uT[:, jf, :],
                                         w2_sb[:, jf, :],
                                         start=(jf == 0), stop=False)
                    nc.tensor.matmul(o2[:], ones_k1[:], b2r_r[:],
                                     start=False, stop=True)
                    x3 = sc3.tile([128, D], f32, tag="x3")
                    nc.vector.tensor_add(x3[:], o2[:], h2[:])
                    o_sb = sc3.tile([128, D], f32, tag="osb")
                    _ln_block(nc, sc3, x3[:], o_sb[:],
                              gb_sb["g3"][:], gb_sb["be3"][:], eps_t[:],
                              tail=nc.gpsimd)
                    nc.sync.dma_start(out_h[rb * 128:(rb + 1) * 128, :], o_sb[:])

            # arrival-ordered issue: ~80us of half-A-only PE work first
            scores_h(0, 0)
            scores_h(1, 0)
            pv_a(0)
            pv_a(1)
            scores_h(0, 1)
            pv_b_ffn(0)
            scores_h(1, 1)
            pv_b_ffn(1)


_NC_CACHE = None


def _get_nc():
    global _NC_CACHE
    if _NC_CACHE is None:
        _NC_CACHE = _build()
    return _NC_CACHE


def _host_prep(inputs):
    """Build per-core in_maps from full inputs."""
    h = np.ascontiguousarray(np.asarray(inputs["h"], dtype=np.float32))
    ei = np.asarray(inputs["edge_index"]).astype(np.int64)
    src, dst = ei[0], ei[1]

    deg = np.bincount(dst, minlength=N).astype(np.float32) + 1.0
    dinv = 1.0 / np.sqrt(deg)
    coef = (dinv[src] * dinv[dst]).astype(np.float32)
    A = np.zeros((N, N), np.float32)
    np.add.at(A, (dst, src), coef)
    idx = np.arange(N)
    A[idx, idx] += dinv * dinv

    f32c = lambda k: np.ascontiguousarray(np.asarray(inputs[k], dtype=np.float32))
    bfc = lambda x: np.ascontiguousarray(x.astype(BF))

    w = {k: f32c(k) for k in ("W_gcn", "Wq", "Wk", "Wv", "Wo", "RF",
                              "W1", "W2", "b_gcn", "bq", "bk", "bv", "bo",
                              "b1", "b2", "g1", "be1", "g2", "be2", "g3", "be3")}

    # h node-major pre-arranged: hn[p, c*D + d] = h[c*128 + p, d]
    hn = np.ascontiguousarray(
        h.reshape(NCH, 128, D).transpose(1, 0, 2).reshape(128, NCH * D)
        .astype(E4))

    # fold the layernorm affines into the consuming projections:
    # (n*g + be) @ W + b == n @ (g[:,None]*W) + (b + be @ W)
    w1f = w["W1"] * w["g2"].reshape(D, 1)
    b1f = w["b1"] + w["be2"] @ w["W1"]
    g1c = w["g1"].reshape(D, 1)
    wqf = w["Wq"] * g1c
    wkf = w["Wk"] * g1c
    wvf = w["Wv"] * g1c
    bqf = w["bq"] + w["be1"] @ w["Wq"]
    bkf = w["bk"] + w["be1"] @ w["Wk"]
    bvf = w["bv"] + w["be1"] @ w["Wv"]

    common = {
        "hn": hn,
        "wgcn": bfc(w["W_gcn"]), "wq": bfc(wqf), "wk": bfc(wkf),
        "wv": bfc(wvf), "wo": bfc(w["Wo"]), "rft": bfc(w["RF"].T),
        "w1": bfc(w1f), "w2": bfc(w["W2"]),
        "bqc": np.ascontiguousarray(bqf.reshape(D, 1)),
        "bkc": np.ascontiguousarray(bkf.reshape(D, 1)),
        "b1c": np.ascontiguousarray(b1f.reshape(F, 1)),
        "bvr": bfc(bvf.reshape(1, D)),
        "bor": bfc(w["bo"].reshape(1, D)),
        "b2r": bfc(w["b2"].reshape(1, D)),
        "g1": bfc(w["g1"].reshape(1, D)),
        "be1": bfc(w["be1"].reshape(1, D)),
        "g2": bfc(w["g2"].reshape(1, D)),
        "be2": bfc(w["be2"].reshape(1, D)),
        "g3": bfc(w["g3"].reshape(1, D)),
        "be3": bfc(w["be3"].reshape(1, D)),
    }

    in_maps = []
    for c in range(NCORES):
        r0 = c * R
        # at[rb, hf, p, k, f] = A[r0 + rb*128 + f, (hf*32+k)*128 + p]
        a_loc = A[r0:r0 + R].reshape(RB, 128, 2, NCH // 2, 128)
        at = np.ascontiguousarray(a_loc.transpose(0, 2, 4, 3, 1).astype(E4))
        hr = (h[r0:r0 + R] + w["b_gcn"]).reshape(RB, 128, D).transpose(
            1, 0, 2).reshape(128, RB * D)
        m = dict(common)
        m["at"] = at
        m["hres"] = np.ascontiguousarray(hr)
        in_maps.append(m)
    return in_maps


def kernel(**inputs):
    nc = _get_nc()
    in_maps = _host_prep(inputs)
    res = run_bass_kernel_spmd(nc, in_maps, core_ids=list(range(NCORES)))
    out = np.concatenate([np.asarray(r["out"]) for r in res.results], axis=0)
    return out.astype(np.float32)

